# revision 62
# baseline (speedup 1.0000x reference)
"""Multi-head self-attention (B=1, S=4096, D=1024, H=16, DK=64) on 8 Trainium2
NeuronCores.

Sharding: tensor(model)-parallel over heads — 2 heads per core. Each core
computes Q^T/K^T/V^T for its 2 heads from the (host-pre-transposed) full x^T,
runs causal flash-style attention fully in transposed space (scores S^T with
keys on partitions, queries on the free dim; softmax sums come free via a
ones-column appended to V), then the per-head outputs are exchanged with
pipelined AllToAlls (bf16 payload) so every core ends up with all 16 heads'
outputs for its own 512-query-row shard, against which it runs the output
projection. The full output is the concatenation of the per-core row shards
(done on host).

The causal mask is structural (reference always builds jnp.tril), so the mask
input is not shipped to the device; masking is done with a precomputed
triangular tile on the diagonal blocks.

All device inputs are pre-arranged on the host so every input DMA is
contiguous per partition (weights as [p, t, m], x^T as [c, p, t, q]).
"""

import numpy as np
from contextlib import ExitStack

import concourse.bass as bass
import concourse.bacc as bacc
import concourse.tile as tile
import concourse.mybir as mybir
from concourse.bass_utils import run_bass_kernel_spmd
from concourse.masks import make_identity

F32 = mybir.dt.float32
F32R = mybir.dt.float32r
BF16 = mybir.dt.bfloat16
EXP = mybir.ActivationFunctionType.Exp
EXPB = -3.0   # exp bias; cancels in the softmax normalization but keeps
              # the unnormalized weights in a bf16-friendly range

N_CORES = 8
D = 1024
H = 16
DK = 64        # head dim
HPC = H // N_CORES          # heads per core (2)
QC = 512                    # query-chunk width (free dim of S^T tiles)


def build(S=4096):
    """Build + compile the SPMD program (identical on all 8 cores)."""
    SC = S // QC            # query chunks
    NSB = S // 128          # 128-wide seq blocks
    QPER = S // N_CORES     # output rows per core

    nc = bacc.Bacc("TRN2", target_bir_lowering=False, debug=False,
                   enable_asserts=False, num_devices=N_CORES)

    # host pre-arranged: xt [c, p, t, q]; w* [p, t, m]; wo [p, t, n]
    xt = nc.dram_tensor("xt", [SC, 128, 8, QC], BF16, kind="ExternalInput")
    wq = nc.dram_tensor("wq", [128, 8, 128], BF16, kind="ExternalInput")
    wk = nc.dram_tensor("wk", [128, 8, 128], BF16, kind="ExternalInput")
    wv = nc.dram_tensor("wv", [128, 8, 128], BF16, kind="ExternalInput")
    wo = nc.dram_tensor("wo", [128, 8, D], BF16, kind="ExternalInput")
    bq = nc.dram_tensor("bq", [128], F32, kind="ExternalInput")
    bk = nc.dram_tensor("bk", [128], F32, kind="ExternalInput")
    bv = nc.dram_tensor("bv", [128], F32, kind="ExternalInput")
    bo = nc.dram_tensor("bo", [D], F32, kind="ExternalInput")
    out = nc.dram_tensor("out", [QPER, D], F32, kind="ExternalOutput")

    with tile.TileContext(nc) as tc, ExitStack() as ctx:
        sb = ctx.enter_context(tc.tile_pool(name="sb", bufs=1))
        sbx = ctx.enter_context(tc.tile_pool(name="sbx", bufs=2))
        sbpt = ctx.enter_context(tc.tile_pool(name="sbpt", bufs=3))
        sbtmp = ctx.enter_context(tc.tile_pool(name="sbtmp", bufs=3))
        # PSUM: one 3-slot pool of [128,1024] tiles (6 banks) shared by all
        # phases + a single [65,1024] accumulator tile (2 banks) = 8 banks.
        ps_big = ctx.enter_context(tc.tile_pool(name="ps_big", bufs=3, space="PSUM"))
        ps_ot = ctx.enter_context(tc.tile_pool(name="ps_ot", bufs=1, space="PSUM"))
        dram = ctx.enter_context(tc.tile_pool(name="dram", bufs=1, space="DRAM"))

        # ---- persistent tensors / constants ------------------------------
        wq_sb = sb.tile([128, 8, 128], BF16)
        wk_sb = sb.tile([128, 8, 128], BF16)
        wv_sb = sb.tile([128, 8, 128], BF16)
        # wq first (the first matmul's critical DMA), split per subtile so
        # matmul t can start as soon as piece t lands; wk/wv are issued
        # after chunk 0's x load below so they don't compete for bandwidth
        for t2 in range(4):
            nc.sync.dma_start(wq_sb[:, 2 * t2: 2 * t2 + 2, :],
                              wq.ap()[:, 2 * t2: 2 * t2 + 2, :])
        bq_sb = sb.tile([128, 1], F32)
        bk_sb = sb.tile([128, 1], F32)
        bv_sb = sb.tile([128, 1], F32)
        nc.sync.dma_start(bq_sb[:], bq.ap().rearrange("(p a) -> p a", a=1))
        nc.sync.dma_start(bk_sb[:], bk.ap().rearrange("(p a) -> p a", a=1))
        nc.sync.dma_start(bv_sb[:], bv.ap().rearrange("(p a) -> p a", a=1))
        bo_bc = sb.tile([128, D], F32)   # bias broadcast along partitions
        wo_sb = sb.tile([128, 8, D], BF16)

        QT = sb.tile([128, S], BF16)      # rows 0-63 head0, 64-127 head1
        KT = sb.tile([128, S], BF16)
        # V' storage per 128-key block: [V_h0 (64) | 1 | V_h1 (64) | 1];
        # the ones columns make the softmax sums fall out of the P.V matmul
        Vp = sb.tile([128, NSB, 130], BF16)
        nc.vector.memset(Vp[:, :, 64:65], 1.0)
        nc.vector.memset(Vp[:, :, 129:130], 1.0)
        expb = sb.tile([128, 1], F32)
        nc.vector.memset(expb[:], EXPB)


        tri_f32 = sb.tile([128, 128], F32)  # tri[pj, j] = 1 if j >= pj else 0
        nc.gpsimd.memset(tri_f32[:], 1.0)
        nc.gpsimd.affine_select(
            out=tri_f32[:], in_=tri_f32[:], compare_op=mybir.AluOpType.is_ge,
            fill=0.0, base=0, pattern=[[1, 128]], channel_multiplier=-1)
        tri = sb.tile([128, 128], BF16)
        nc.vector.tensor_copy(tri[:], tri_f32[:])
        ident = sb.tile([128, 128], F32)
        make_identity(nc, ident[:])

        # Output ownership is interleaved so the AllToAll can be split into
        # pipelined exchanges. Group g covers chunk range GROUPS[g]; within
        # its row span rank r owns an interleaved GW-wide slice. A2A #g
        # fires as soon as the group's chunks are staged and overlaps the
        # remaining attention chunks. The last two groups are single chunks
        # so the final (exposed) exchange is half-sized. Payload: rows
        # 0-127 = producer-normalized O^T (h0, h1) in bf16.
        GROUPS = [(0, 2), (2, 4), (4, 6), (6, 7), (7, 8)]
        NG = len(GROUPS)
        GWS = [(hi - lo) * QC // N_CORES for lo, hi in GROUPS]
        ROWB = [sum(GWS[:g]) for g in range(NG)]     # out row base per group
        CHUNK_G = {}
        for g, (lo, hi) in enumerate(GROUPS):
            for c in range(lo, hi):
                CHUNK_G[c] = g
        a2a_in = [dram.tile([N_CORES, 128, GWS[g]], BF16, name=f"a2ain{g}")
                  for g in range(NG)]
        a2a_out = [dram.tile([N_CORES, 128, GWS[g]], BF16, name=f"a2aout{g}")
                   for g in range(NG)]

        # tiny warm-up exchange: absorbs the communicator-init barrier and
        # first-collective overhead while the early QKV chunks compute
        warm_in = dram.tile([N_CORES, 32], F32)
        warm_out = dram.tile([N_CORES, 32], F32)
        nc.gpsimd.collective_compute(
            "AllToAll", mybir.AluOpType.bypass,
            replica_groups=[list(range(N_CORES))],
            ins=[warm_in.opt()], outs=[warm_out.opt()])

        def make_qkv_bursts(c):
            """Per-chunk QKV work as small PE bursts. Interleaved between
            attention pairs of the previous chunk, they fill what would be
            PE idle time."""
            xt_sb = sbx.tile([128, 8, QC], BF16, tag="xt", name=f"xt{c}")
            if c == 0:
                # split the first load across queues so the first projection
                # matmul can start as soon as its subtile lands
                for t2 in range(4):
                    nc.sync.dma_start(xt_sb[:, 2 * t2: 2 * t2 + 2, :],
                                      xt.ap()[c, :, 2 * t2: 2 * t2 + 2, :])
                # now the deferred startup loads (off the critical path)
                for w_sb, w in ((wk_sb, wk), (wv_sb, wv)):
                    nc.sync.dma_start(w_sb[:, 0:4, :], w.ap()[:, 0:4, :])
                    nc.sync.dma_start(w_sb[:, 4:8, :], w.ap()[:, 4:8, :])
            else:
                nc.sync.dma_start(xt_sb[:], xt.ap()[c])
            cs = slice(c * QC, (c + 1) * QC)
            st8 = {}

            def proj_burst(w_sb, b_sb, dst):
                def run():
                    p_ps = ps_big.tile([128, 1024], F32, tag="st",
                                       name=f"qkv{c}_{dst.name}")
                    for t in range(8):
                        nc.tensor.matmul(p_ps[:, 0:512], w_sb[:, t, :],
                                         xt_sb[:, t, :],
                                         start=(t == 0), stop=(t == 7))
                    nc.vector.tensor_scalar_add(dst, p_ps[:, 0:512], b_sb[:])
                return run

            def q_burst():
                proj_burst(wq_sb, bq_sb, QT[:, cs])()
            def k_burst():
                proj_burst(wk_sb, bk_sb, KT[:, cs])()
            def v_burst():
                vt_sb = sbtmp.tile([128, QC], F32, tag="vt", name=f"vt{c}")
                st8["vt"] = vt_sb
                proj_burst(wv_sb, bv_sb, vt_sb[:])()

            def t_burst(sbk):
                def run():
                    blk = c * 4 + sbk
                    vt_sb = st8["vt"]
                    tp_ps = ps_big.tile([128, 128], F32, tag="st",
                                        name=f"tp{blk}")
                    nc.tensor.transpose(
                        tp_ps[:], vt_sb[:, sbk * 128:(sbk + 1) * 128], ident[:])
                    nc.vector.tensor_copy(Vp[:, blk, 0:64], tp_ps[:, 0:64])
                    nc.vector.tensor_copy(Vp[:, blk, 65:129],
                                          tp_ps[:, 64:128])
                return run

            return [q_burst, k_burst, v_burst,
                    t_burst(0), t_burst(1), t_burst(2), t_burst(3)]

        def emit_group_output(g):
            # payload arrives already normalized; gather split across 4
            # DMA queues: ofb[p, s, q] = a2a_out[g][s, p, q]
            gw = GWS[g]
            ofb = sbtmp.tile([128, 8, gw], BF16, tag="ofb", name=f"ofb{g}")
            # one DMA per source rank so the first projection matmul can
            # start as soon as its piece lands
            for s in range(8):
                nc.sync.dma_start(
                    ofb[:, s: s + 1, :],
                    a2a_out[g][s: s + 1, :, :].rearrange("s p q -> p s q"))
            for m in range(max(1, gw // 128)):
                mw = min(gw, 128)
                for n2 in range(D // 512):
                    op_ps = ps_big.tile([128, 512], F32, tag="st",
                                        name=f"op{g}_{m}_{n2}")
                    for s in range(8):
                        nc.tensor.matmul(
                            op_ps[0:mw, :], ofb[:, s, m * 128: m * 128 + mw],
                            wo_sb[:, s, n2 * 512:(n2 + 1) * 512],
                            start=(s == 0), stop=(s == 7))
                    o_sb = sbtmp.tile([mw, 512], F32, tag="osb",
                                      name=f"o{g}_{m}_{n2}")
                    nc.vector.tensor_add(o_sb[:], op_ps[0:mw, :],
                                         bo_bc[0:mw, n2 * 512:(n2 + 1) * 512])
                    nc.sync.dma_start(
                        out.ap()[ROWB[g] + m * 128: ROWB[g] + m * 128 + mw,
                                 n2 * 512:(n2 + 1) * 512],
                        o_sb[:])


        for b in make_qkv_bursts(0):
            b()
        for c in range(SC):
            pending = make_qkv_bursts(c + 1) if c + 1 < SC else []
            nb = len(pending)
            done = 0

            # ---- causal attention for chunk c, both heads ----------------
            cs = slice(c * QC, (c + 1) * QC)
            nkb = 4 * (c + 1)
            npairs = nkb // 2
            ot = ps_ot.tile([128, 1024], F32, tag="ot", name=f"ot{c}")
            ots = [ot[0:65, 0:512], ot[0:65, 512:1024]]
            for p, kbp in enumerate(range(0, nkb, 2)):
                st_h = [ps_big.tile([128, 1024], F32, tag="st",
                                    name=f"st{c}_{kbp}_{h}") for h in range(2)]
                # per-block causal offset: queries below 128*t are fully
                # masked for diagonal block t — skip their score columns
                offs = []
                for j in range(2):
                    t = kbp + j - 4 * c
                    offs.append(128 * t if t > 0 else 0)
                # heads interleaved: their PE row-groups (0-63 / 64-127)
                # execute concurrently in the array
                for j in range(2):
                    kb = kbp + j
                    for h in range(2):
                        hs = slice(h * 64, (h + 1) * 64)
                        nc.tensor.matmul(
                            st_h[h][:, j * 512 + offs[j]:(j + 1) * 512],
                            KT[hs, kb * 128:(kb + 1) * 128],
                            QT[hs, c * QC + offs[j]:(c + 1) * QC],
                            start=True, stop=True)
                pt_h = []
                for h in range(2):
                    pt = sbpt.tile([128, 2, 512], BF16,
                                   tag="pt", name=f"pt{c}_{kbp}_{h}")
                    if offs[0] >= 256:
                        # big skipped prefixes: exp each block separately
                        for j in range(2):
                            nc.scalar.activation(
                                pt[:, j, offs[j]:512],
                                st_h[h][:, j * 512 + offs[j]:(j + 1) * 512],
                                EXP, bias=expb[:], scale=0.125)
                    else:
                        nc.scalar.activation(
                            pt[:].rearrange("p j n -> p (j n)"), st_h[h][:],
                            EXP, bias=expb[:], scale=0.125)
                    for j in range(2):
                        t = kbp + j - 4 * c
                        if t >= 0:   # diagonal block: apply causal mask
                            ms = slice(128 * t, 128 * t + 128)
                            nc.vector.tensor_mul(pt[:, j, ms], pt[:, j, ms],
                                                 tri[:])
                    pt_h.append(pt)
                for h in range(2):
                    for j in range(2):
                        kb = kbp + j
                        nc.tensor.matmul(
                            ots[h][:, offs[j]:512],
                            Vp[:, kb, h * 65:(h + 1) * 65],
                            pt_h[h][:, j, offs[j]:512],
                            start=(kb == 0), stop=(kb == nkb - 1))
                # spread next chunk's QKV bursts across this chunk's pairs
                want = (p + 1) * nb // npairs
                while done < want:
                    pending[done]()
                    done += 1
            while done < nb:
                pending[done]()
                done += 1
            # normalize on the producer: copy the sums row out of PSUM,
            # broadcast it down 64 partitions with a 1-row f32r matmul,
            # reciprocal at full lane width, then one multiply casts the
            # normalized O^T to bf16; stage with 2 descriptors.
            g = CHUNK_G[c]
            lo, hi = GROUPS[g]
            jj = c - lo
            gw = GWS[g]
            # copy O^T+sums out of PSUM first so the ot accumulator frees
            # for the next chunk's P.V immediately; then broadcast the raw
            # sums down 64 partitions (idle GpSimd) and reciprocal at full
            # lane width on the DVE
            # sums copy first so the GpSimd broadcast overlaps the O^T copy
            s1 = sbtmp.tile([1, 1024], F32, tag="s1", name=f"s1_{c}")
            nc.vector.tensor_copy(s1[:], ot[64:65, :])
            rb = sbtmp.tile([64, 1024], F32, tag="rb", name=f"rb{c}")
            nc.gpsimd.partition_broadcast(rb[:], s1[:])
            on_f = sbtmp.tile([64, 1024], F32, tag="onf", name=f"onf{c}")
            nc.vector.tensor_copy(on_f[:], ot[0:64, :])
            nc.vector.reciprocal_approx_fast(rb[:], rb[:])
            on_sb = sbtmp.tile([64, 1024], BF16, tag="on", name=f"on{c}")
            nc.vector.tensor_mul(on_sb[:], on_f[:], rb[:])
            npc = QC // gw          # owner pieces per chunk
            dma_eng = nc.scalar if c == SC - 1 else nc.sync
            for h in range(2):
                dma_eng.dma_start(
                    a2a_in[g][npc * jj: npc * (jj + 1), h * 64:(h + 1) * 64, :]
                    .rearrange("i p q -> p i q"),
                    on_sb[:, h * QC:(h + 1) * QC]
                    .rearrange("p (i q) -> p i q", i=npc))

            if c == 1:
                nc.sync.dma_start(
                    bo_bc[:],
                    bo.ap().rearrange("(a n) -> a n", a=1)
                    .to_broadcast((128, D)))
                nc.sync.dma_start(wo_sb[:], wo.ap())
            if c == hi - 1:
                # ---- exchange group g; overlaps later attention chunks ---
                nc.gpsimd.collective_compute(
                    "AllToAll", mybir.AluOpType.bypass,
                    replica_groups=[list(range(N_CORES))],
                    ins=[a2a_in[g].opt()], outs=[a2a_out[g].opt()])
            # all earlier groups' projections are held back and emitted
            # after the final exchange fires, so the PE chews through them
            # while it flies instead of idling
            for ge in {7: [0, 1, 2, 3]}.get(c, []):
                emit_group_output(ge)

        emit_group_output(NG - 1)

    nc.compile()
    return nc


_NC_CACHE = {}


def _get_nc(S):
    if S not in _NC_CACHE:
        _NC_CACHE[S] = build(S)
    return _NC_CACHE[S]


def kernel(x, mask, Wq, bq, Wk, bk, Wv, bv, Wo, bo):
    import ml_dtypes
    x = np.asarray(x, np.float32)
    S = x.shape[1]
    SC = S // QC
    xt = np.ascontiguousarray(x[0].T).astype(ml_dtypes.bfloat16)  # [D, S]
    # [c, p, t, q] layout so the per-chunk DMA is contiguous
    xt_pre = np.ascontiguousarray(
        xt.reshape(8, 128, SC, QC).transpose(2, 1, 0, 3))
    Wq, Wk, Wv, Wo = (np.asarray(w, np.float32) for w in (Wq, Wk, Wv, Wo))
    bq, bk, bv, bo = (np.asarray(b, np.float32) for b in (bq, bk, bv, bo))
    wo_pre = np.ascontiguousarray(
        Wo.reshape(8, 128, D).transpose(1, 0, 2)).astype(ml_dtypes.bfloat16)
    # mask is structurally causal (jnp.tril in the reference); handled on-device.

    in_maps = []
    for r in range(N_CORES):
        sl = slice(128 * r, 128 * (r + 1))
        def wpre(W):
            return np.ascontiguousarray(
                W[:, sl].reshape(8, 128, 128).transpose(1, 0, 2)
            ).astype(ml_dtypes.bfloat16)
        in_maps.append({
            "xt": xt_pre,
            "wq": wpre(Wq),
            "wk": wpre(Wk),
            "wv": wpre(Wv),
            "wo": wo_pre,
            "bq": np.ascontiguousarray(bq[sl]),
            "bk": np.ascontiguousarray(bk[sl]),
            "bv": np.ascontiguousarray(bv[sl]),
            "bo": bo,
        })
    nc = _get_nc(S)
    global LAST_RESULT
    LAST_RESULT = run_bass_kernel_spmd(nc, in_maps, list(range(N_CORES)),
                                       trace=TRACE)
    res = LAST_RESULT.results
    # group g's shard rows of rank r hold global rows
    # QC*lo + GW_g*r + [0, GW_g)
    GROUPS = [(0, 2), (2, 4), (4, 6), (6, 7), (7, 8)]
    GWS = [(hi - lo) * QC // N_CORES for lo, hi in GROUPS]
    full = np.empty((S, D), np.float32)
    for r in range(N_CORES):
        o = res[r]["out"]
        rb = 0
        for (lo, hi), gw in zip(GROUPS, GWS):
            full[QC * lo + gw * r: QC * lo + gw * (r + 1)] = o[rb: rb + gw]
            rb += gw
    return full[None]


TRACE = False          # test harness flips this to profile
LAST_RESULT = None


# revision 66
# speedup vs baseline: 1.0518x; 1.0518x over previous
"""Multi-head self-attention (B=1, S=4096, D=1024, H=16, DK=64) on 8 Trainium2
NeuronCores.

Sharding: tensor(model)-parallel over heads — 2 heads per core. Each core
computes Q^T/K^T/V^T for its 2 heads from the (host-pre-transposed) full x^T,
runs causal flash-style attention fully in transposed space (scores S^T with
keys on partitions, queries on the free dim; softmax sums come free via a
ones-column appended to V), then the per-head outputs are exchanged with
pipelined AllToAlls (bf16 payload) so every core ends up with all 16 heads'
outputs for its own 512-query-row shard, against which it runs the output
projection. The full output is the concatenation of the per-core row shards
(done on host).

The causal mask is structural (reference always builds jnp.tril), so the mask
input is not shipped to the device; masking is done with a precomputed
triangular tile on the diagonal blocks.

All device inputs are pre-arranged on the host so every input DMA is
contiguous per partition (weights as [p, t, m], x^T as [c, p, t, q]).
"""

import numpy as np
from contextlib import ExitStack

import concourse.bass as bass
import concourse.bacc as bacc
import concourse.tile as tile
import concourse.mybir as mybir
from concourse.bass_utils import run_bass_kernel_spmd
from concourse.masks import make_identity

F32 = mybir.dt.float32
F32R = mybir.dt.float32r
BF16 = mybir.dt.bfloat16
EXP = mybir.ActivationFunctionType.Exp
EXPB = -3.0   # exp bias; cancels in the softmax normalization but keeps
              # the unnormalized weights in a bf16-friendly range

N_CORES = 8
D = 1024
H = 16
DK = 64        # head dim
HPC = H // N_CORES          # heads per core (2)
QC = 512                    # query-chunk width (free dim of S^T tiles)


def build(S=4096):
    """Build + compile the SPMD program (identical on all 8 cores)."""
    SC = S // QC            # query chunks
    NSB = S // 128          # 128-wide seq blocks
    QPER = S // N_CORES     # output rows per core

    nc = bacc.Bacc("TRN2", target_bir_lowering=False, debug=False,
                   enable_asserts=False, num_devices=N_CORES)

    # host pre-arranged: xt [c, p, t, q]; w* [p, t, m]; wo [p, t, n]
    xt = nc.dram_tensor("xt", [SC, 128, 8, QC], BF16, kind="ExternalInput")
    wq = nc.dram_tensor("wq", [128, 8, 128], BF16, kind="ExternalInput")
    wk = nc.dram_tensor("wk", [128, 8, 128], BF16, kind="ExternalInput")
    wv = nc.dram_tensor("wv", [128, 8, 128], BF16, kind="ExternalInput")
    wo = nc.dram_tensor("wo", [128, 8, D], BF16, kind="ExternalInput")
    bq = nc.dram_tensor("bq", [128], F32, kind="ExternalInput")
    bk = nc.dram_tensor("bk", [128], F32, kind="ExternalInput")
    bv = nc.dram_tensor("bv", [128], F32, kind="ExternalInput")
    bo = nc.dram_tensor("bo", [D], BF16, kind="ExternalInput")
    out = nc.dram_tensor("out", [QPER, D], F32, kind="ExternalOutput")

    with tile.TileContext(nc) as tc, ExitStack() as ctx:
        sb = ctx.enter_context(tc.tile_pool(name="sb", bufs=1))
        sbx = ctx.enter_context(tc.tile_pool(name="sbx", bufs=2))
        sbpt = ctx.enter_context(tc.tile_pool(name="sbpt", bufs=3))
        sbtmp = ctx.enter_context(tc.tile_pool(name="sbtmp", bufs=3))
        # PSUM: one 3-slot pool of [128,1024] tiles (6 banks) shared by all
        # phases + a single [65,1024] accumulator tile (2 banks) = 8 banks.
        ps_big = ctx.enter_context(tc.tile_pool(name="ps_big", bufs=3, space="PSUM"))
        ps_ot = ctx.enter_context(tc.tile_pool(name="ps_ot", bufs=1, space="PSUM"))
        dram = ctx.enter_context(tc.tile_pool(name="dram", bufs=1, space="DRAM"))

        # ---- persistent tensors / constants ------------------------------
        wq_sb = sb.tile([128, 8, 128], BF16)
        wk_sb = sb.tile([128, 8, 128], BF16)
        wv_sb = sb.tile([128, 8, 128], BF16)
        # wq first (the first matmul's critical DMA), split per subtile so
        # matmul t can start as soon as piece t lands; wk/wv are issued
        # after chunk 0's x load below so they don't compete for bandwidth
        for t2 in range(4):
            nc.sync.dma_start(wq_sb[:, 2 * t2: 2 * t2 + 2, :],
                              wq.ap()[:, 2 * t2: 2 * t2 + 2, :])
        bq_sb = sb.tile([128, 1], F32)
        bk_sb = sb.tile([128, 1], F32)
        bv_sb = sb.tile([128, 1], F32)
        nc.sync.dma_start(bq_sb[:], bq.ap().rearrange("(p a) -> p a", a=1))
        nc.sync.dma_start(bk_sb[:], bk.ap().rearrange("(p a) -> p a", a=1))
        nc.sync.dma_start(bv_sb[:], bv.ap().rearrange("(p a) -> p a", a=1))
        bo1 = sb.tile([1, D], BF16)      # output bias, added via a 1-row
        ones1 = sb.tile([1, 128], BF16)  # matmul so the emit epilogue
        nc.vector.memset(ones1[:], 1.0)  # stays off the DVE queue
        wo_sb = sb.tile([128, 8, D], BF16)

        QT = sb.tile([128, S], BF16)      # rows 0-63 head0, 64-127 head1
        KT = sb.tile([128, S], BF16)
        # V' storage per 128-key block: [V_h0 (64) | 1 | V_h1 (64) | 1];
        # the ones columns make the softmax sums fall out of the P.V matmul
        Vp = sb.tile([128, NSB, 130], BF16)
        nc.vector.memset(Vp[:, :, 64:65], 1.0)
        nc.vector.memset(Vp[:, :, 129:130], 1.0)
        expb = sb.tile([128, 1], F32)
        nc.vector.memset(expb[:], EXPB)


        tri_f32 = sb.tile([128, 128], F32)  # tri[pj, j] = 1 if j >= pj else 0
        nc.gpsimd.memset(tri_f32[:], 1.0)
        nc.gpsimd.affine_select(
            out=tri_f32[:], in_=tri_f32[:], compare_op=mybir.AluOpType.is_ge,
            fill=0.0, base=0, pattern=[[1, 128]], channel_multiplier=-1)
        tri = sb.tile([128, 128], BF16)
        nc.vector.tensor_copy(tri[:], tri_f32[:])
        ident = sb.tile([128, 128], F32)
        make_identity(nc, ident[:])

        # Output ownership is interleaved so the AllToAll can be split into
        # pipelined exchanges. Group g covers chunk range GROUPS[g]; within
        # its row span rank r owns an interleaved GW-wide slice. A2A #g
        # fires as soon as the group's chunks are staged and overlaps the
        # remaining attention chunks. The last two groups are single chunks
        # so the final (exposed) exchange is half-sized. Payload: rows
        # 0-127 = producer-normalized O^T (h0, h1) in bf16.
        GROUPS = [(0, 2), (2, 4), (4, 6), (6, 7), (7, 8)]
        NG = len(GROUPS)
        GWS = [(hi - lo) * QC // N_CORES for lo, hi in GROUPS]
        ROWB = [sum(GWS[:g]) for g in range(NG)]     # out row base per group
        CHUNK_G = {}
        for g, (lo, hi) in enumerate(GROUPS):
            for c in range(lo, hi):
                CHUNK_G[c] = g
        a2a_in = [dram.tile([N_CORES, 128, GWS[g]], BF16, name=f"a2ain{g}")
                  for g in range(NG)]
        a2a_out = [dram.tile([N_CORES, 128, GWS[g]], BF16, name=f"a2aout{g}")
                   for g in range(NG)]

        # tiny warm-up exchange: absorbs the communicator-init barrier and
        # first-collective overhead while the early QKV chunks compute
        warm_in = dram.tile([N_CORES, 32], F32)
        warm_out = dram.tile([N_CORES, 32], F32)
        nc.gpsimd.collective_compute(
            "AllToAll", mybir.AluOpType.bypass,
            replica_groups=[list(range(N_CORES))],
            ins=[warm_in.opt()], outs=[warm_out.opt()])

        def make_qkv_bursts(c):
            """Per-chunk QKV work as small PE bursts. Interleaved between
            attention pairs of the previous chunk, they fill what would be
            PE idle time."""
            xt_sb = sbx.tile([128, 8, QC], BF16, tag="xt", name=f"xt{c}")
            if c == 0:
                # split the first load across queues so the first projection
                # matmul can start as soon as its subtile lands
                for t2 in range(4):
                    nc.sync.dma_start(xt_sb[:, 2 * t2: 2 * t2 + 2, :],
                                      xt.ap()[c, :, 2 * t2: 2 * t2 + 2, :])
                # now the deferred startup loads (off the critical path)
                for w_sb, w in ((wk_sb, wk), (wv_sb, wv)):
                    nc.sync.dma_start(w_sb[:, 0:4, :], w.ap()[:, 0:4, :])
                    nc.sync.dma_start(w_sb[:, 4:8, :], w.ap()[:, 4:8, :])
            else:
                nc.sync.dma_start(xt_sb[:], xt.ap()[c])
            cs = slice(c * QC, (c + 1) * QC)
            st8 = {}

            def proj_burst(w_sb, b_sb, dst):
                def run():
                    p_ps = ps_big.tile([128, 1024], F32, tag="st",
                                       name=f"qkv{c}_{dst.name}")
                    for t in range(8):
                        nc.tensor.matmul(p_ps[:, 0:512], w_sb[:, t, :],
                                         xt_sb[:, t, :],
                                         start=(t == 0), stop=(t == 7))
                    nc.vector.tensor_scalar_add(dst, p_ps[:, 0:512], b_sb[:])
                return run

            def q_burst():
                proj_burst(wq_sb, bq_sb, QT[:, cs])()
            def k_burst():
                proj_burst(wk_sb, bk_sb, KT[:, cs])()
            def v_burst():
                vt_sb = sbtmp.tile([128, QC], F32, tag="vt", name=f"vt{c}")
                st8["vt"] = vt_sb
                proj_burst(wv_sb, bv_sb, vt_sb[:])()

            def t_burst(sbk):
                def run():
                    blk = c * 4 + sbk
                    vt_sb = st8["vt"]
                    tp_ps = ps_big.tile([128, 128], F32, tag="st",
                                        name=f"tp{blk}")
                    nc.tensor.transpose(
                        tp_ps[:], vt_sb[:, sbk * 128:(sbk + 1) * 128], ident[:])
                    nc.vector.tensor_copy(Vp[:, blk, 0:64], tp_ps[:, 0:64])
                    nc.vector.tensor_copy(Vp[:, blk, 65:129],
                                          tp_ps[:, 64:128])
                return run

            return [q_burst, k_burst, v_burst,
                    t_burst(0), t_burst(1), t_burst(2), t_burst(3)]

        def emit_group_output(g):
            # payload arrives already normalized; gather split across 4
            # DMA queues: ofb[p, s, q] = a2a_out[g][s, p, q]
            gw = GWS[g]
            ofb = sbtmp.tile([128, 8, gw], BF16, tag="ofb", name=f"ofb{g}")
            # one DMA per source rank so the first projection matmul can
            # start as soon as its piece lands
            for s in range(8):
                nc.sync.dma_start(
                    ofb[:, s: s + 1, :],
                    a2a_out[g][s: s + 1, :, :].rearrange("s p q -> p s q"))
            for m in range(max(1, gw // 128)):
                mw = min(gw, 128)
                for n2 in range(D // 512):
                    op_ps = ps_big.tile([128, 512], F32, tag="st",
                                        name=f"op{g}_{m}_{n2}")
                    for s in range(8):
                        nc.tensor.matmul(
                            op_ps[0:mw, :], ofb[:, s, m * 128: m * 128 + mw],
                            wo_sb[:, s, n2 * 512:(n2 + 1) * 512],
                            start=(s == 0), stop=False)
                    nc.tensor.matmul(
                        op_ps[0:mw, :], ones1[0:1, 0:mw],
                        bo1[0:1, n2 * 512:(n2 + 1) * 512],
                        start=False, stop=True)
                    # PSUM -> SBUF via the Scalar engine (idle at the tail)
                    # so the normalize chain on the DVE is not interleaved
                    o_sb = sbtmp.tile([mw, 512], F32, tag="osb",
                                      name=f"o{g}_{m}_{n2}")
                    nc.scalar.activation(o_sb[:], op_ps[0:mw, :],
                                         mybir.ActivationFunctionType.Copy)
                    nc.sync.dma_start(
                        out.ap()[ROWB[g] + m * 128: ROWB[g] + m * 128 + mw,
                                 n2 * 512:(n2 + 1) * 512],
                        o_sb[:])


        for b in make_qkv_bursts(0):
            b()
        for c in range(SC):
            pending = make_qkv_bursts(c + 1) if c + 1 < SC else []
            nb = len(pending)
            done = 0

            # ---- causal attention for chunk c, both heads ----------------
            cs = slice(c * QC, (c + 1) * QC)
            nkb = 4 * (c + 1)
            npairs = nkb // 2
            ot = ps_ot.tile([128, 1024], F32, tag="ot", name=f"ot{c}")
            ots = [ot[0:65, 0:512], ot[0:65, 512:1024]]
            for p, kbp in enumerate(range(0, nkb, 2)):
                st_h = [ps_big.tile([128, 1024], F32, tag="st",
                                    name=f"st{c}_{kbp}_{h}") for h in range(2)]
                # per-block causal offset: queries below 128*t are fully
                # masked for diagonal block t — skip their score columns
                offs = []
                for j in range(2):
                    t = kbp + j - 4 * c
                    offs.append(128 * t if t > 0 else 0)
                # heads interleaved: their PE row-groups (0-63 / 64-127)
                # execute concurrently in the array
                for j in range(2):
                    kb = kbp + j
                    for h in range(2):
                        hs = slice(h * 64, (h + 1) * 64)
                        nc.tensor.matmul(
                            st_h[h][:, j * 512 + offs[j]:(j + 1) * 512],
                            KT[hs, kb * 128:(kb + 1) * 128],
                            QT[hs, c * QC + offs[j]:(c + 1) * QC],
                            start=True, stop=True)
                pt_h = []
                for h in range(2):
                    pt = sbpt.tile([128, 2, 512], BF16,
                                   tag="pt", name=f"pt{c}_{kbp}_{h}")
                    if offs[0] >= 256:
                        # big skipped prefixes: exp each block separately
                        for j in range(2):
                            nc.scalar.activation(
                                pt[:, j, offs[j]:512],
                                st_h[h][:, j * 512 + offs[j]:(j + 1) * 512],
                                EXP, bias=expb[:], scale=0.125)
                    else:
                        nc.scalar.activation(
                            pt[:].rearrange("p j n -> p (j n)"), st_h[h][:],
                            EXP, bias=expb[:], scale=0.125)
                    for j in range(2):
                        t = kbp + j - 4 * c
                        if t >= 0:   # diagonal block: apply causal mask
                            ms = slice(128 * t, 128 * t + 128)
                            nc.vector.tensor_mul(pt[:, j, ms], pt[:, j, ms],
                                                 tri[:])
                    pt_h.append(pt)
                for h in range(2):
                    for j in range(2):
                        kb = kbp + j
                        nc.tensor.matmul(
                            ots[h][:, offs[j]:512],
                            Vp[:, kb, h * 65:(h + 1) * 65],
                            pt_h[h][:, j, offs[j]:512],
                            start=(kb == 0), stop=(kb == nkb - 1))
                # spread next chunk's QKV bursts across this chunk's pairs
                want = (p + 1) * nb // npairs
                while done < want:
                    pending[done]()
                    done += 1
            while done < nb:
                pending[done]()
                done += 1
            # normalize on the producer: copy the sums row out of PSUM,
            # broadcast it down 64 partitions with a 1-row f32r matmul,
            # reciprocal at full lane width, then one multiply casts the
            # normalized O^T to bf16; stage with 2 descriptors.
            g = CHUNK_G[c]
            lo, hi = GROUPS[g]
            jj = c - lo
            gw = GWS[g]
            # copy O^T+sums out of PSUM first so the ot accumulator frees
            # for the next chunk's P.V immediately; then broadcast the raw
            # sums down 64 partitions (idle GpSimd) and reciprocal at full
            # lane width on the DVE
            # sums copy first so the GpSimd broadcast overlaps the O^T copy
            s1 = sbtmp.tile([1, 1024], F32, tag="s1", name=f"s1_{c}")
            nc.vector.tensor_copy(s1[:], ot[64:65, :])
            rb = sbtmp.tile([64, 1024], F32, tag="rb", name=f"rb{c}")
            nc.gpsimd.partition_broadcast(rb[:], s1[:])
            on_f = sbtmp.tile([64, 1024], F32, tag="onf", name=f"onf{c}")
            nc.vector.tensor_copy(on_f[:], ot[0:64, :])
            nc.vector.reciprocal_approx_fast(rb[:], rb[:])
            on_sb = sbtmp.tile([64, 1024], BF16, tag="on", name=f"on{c}")
            nc.vector.tensor_mul(on_sb[:], on_f[:], rb[:])
            npc = QC // gw          # owner pieces per chunk
            dma_eng = nc.scalar if c == SC - 1 else nc.sync
            for h in range(2):
                dma_eng.dma_start(
                    a2a_in[g][npc * jj: npc * (jj + 1), h * 64:(h + 1) * 64, :]
                    .rearrange("i p q -> p i q"),
                    on_sb[:, h * QC:(h + 1) * QC]
                    .rearrange("p (i q) -> p i q", i=npc))

            if c == 1:
                nc.sync.dma_start(bo1[:],
                                  bo.ap().rearrange("(a n) -> a n", a=1))
                nc.sync.dma_start(wo_sb[:], wo.ap())
            if c == hi - 1:
                # ---- exchange group g; overlaps later attention chunks ---
                nc.gpsimd.collective_compute(
                    "AllToAll", mybir.AluOpType.bypass,
                    replica_groups=[list(range(N_CORES))],
                    ins=[a2a_in[g].opt()], outs=[a2a_out[g].opt()])
            # all earlier groups' projections are held back and emitted
            # after the final exchange fires, so the PE chews through them
            # while it flies instead of idling
            for ge in {7: [0, 1, 2, 3]}.get(c, []):
                emit_group_output(ge)

        emit_group_output(NG - 1)

    nc.compile()
    return nc


_NC_CACHE = {}


def _get_nc(S):
    if S not in _NC_CACHE:
        _NC_CACHE[S] = build(S)
    return _NC_CACHE[S]


def kernel(x, mask, Wq, bq, Wk, bk, Wv, bv, Wo, bo):
    import ml_dtypes
    x = np.asarray(x, np.float32)
    S = x.shape[1]
    SC = S // QC
    xt = np.ascontiguousarray(x[0].T).astype(ml_dtypes.bfloat16)  # [D, S]
    # [c, p, t, q] layout so the per-chunk DMA is contiguous
    xt_pre = np.ascontiguousarray(
        xt.reshape(8, 128, SC, QC).transpose(2, 1, 0, 3))
    Wq, Wk, Wv, Wo = (np.asarray(w, np.float32) for w in (Wq, Wk, Wv, Wo))
    bq, bk, bv, bo = (np.asarray(b, np.float32) for b in (bq, bk, bv, bo))
    wo_pre = np.ascontiguousarray(
        Wo.reshape(8, 128, D).transpose(1, 0, 2)).astype(ml_dtypes.bfloat16)
    # mask is structurally causal (jnp.tril in the reference); handled on-device.

    in_maps = []
    for r in range(N_CORES):
        sl = slice(128 * r, 128 * (r + 1))
        def wpre(W):
            return np.ascontiguousarray(
                W[:, sl].reshape(8, 128, 128).transpose(1, 0, 2)
            ).astype(ml_dtypes.bfloat16)
        in_maps.append({
            "xt": xt_pre,
            "wq": wpre(Wq),
            "wk": wpre(Wk),
            "wv": wpre(Wv),
            "wo": wo_pre,
            "bq": np.ascontiguousarray(bq[sl]),
            "bk": np.ascontiguousarray(bk[sl]),
            "bv": np.ascontiguousarray(bv[sl]),
            "bo": bo.astype(ml_dtypes.bfloat16),
        })
    nc = _get_nc(S)
    global LAST_RESULT
    LAST_RESULT = run_bass_kernel_spmd(nc, in_maps, list(range(N_CORES)),
                                       trace=TRACE)
    res = LAST_RESULT.results
    # group g's shard rows of rank r hold global rows
    # QC*lo + GW_g*r + [0, GW_g)
    GROUPS = [(0, 2), (2, 4), (4, 6), (6, 7), (7, 8)]
    GWS = [(hi - lo) * QC // N_CORES for lo, hi in GROUPS]
    full = np.empty((S, D), np.float32)
    for r in range(N_CORES):
        o = res[r]["out"]
        rb = 0
        for (lo, hi), gw in zip(GROUPS, GWS):
            full[QC * lo + gw * r: QC * lo + gw * (r + 1)] = o[rb: rb + gw]
            rb += gw
    return full[None]


TRACE = False          # test harness flips this to profile
LAST_RESULT = None
